# revision 40
# baseline (speedup 1.0000x reference)
"""CenterLoss Trainium2 kernel (8 NeuronCores, data-parallel over batch).

Math: the reference builds the full [N, C] masked distance matrix, but only
the labeled entry of each row survives the mask, so

    loss = ( sum_i ||x_i - centers[labels_i]||^2  +  N*(C-1)*CLAMP_MIN ) / N

(the second term is the clamp applied to the zeroed-out entries). Expanding
||x_i - c||^2 = ||x_i||^2 - 2 x_i.c + ||c||^2:

    sum_i d_i = sum(x*x) + sum_c n_c ||c_c||^2 - 2 sum_i x_i . c_{l_i}

Band strategy (v5): the host sorts each core's 2048 samples by label, so
each 128-sample tile's labels span < 128 consecutive centers (measured max
span 79 for this problem size). The cross term then only needs a [128, 128]
BAND of the x @ centers^T product per tile — 16 small PE matmuls instead of
a per-sample gather (which is Q7-descriptor-bound at ~1us per 128 rows).
The labeled entry of each band row is selected with an on-device one-hot
(iota == rel) and a fused multiply-reduce (DVE tensor_tensor_reduce).
n_c (the label histogram, metadata derived from labels only) is computed
host-side; ||c||^2 is computed on device by ACT square-accumulate.

Host prep is layout-only: sort/transpose/slice of inputs + label metadata.
All arithmetic on x and centers happens on device. Falls back to the v4
indirect-DMA gather kernel if any tile span exceeds the band width.
"""

import numpy as np

import concourse.bacc as bacc
import concourse.tile as tile
from concourse import bass, mybir
from concourse.bass_utils import run_bass_kernel_spmd

N, C, D = 16384, 1024, 128
N_CORES = 8
NS = N // N_CORES  # 2048 samples per core
P = 128
T = NS // P  # 16 tiles per core
W = 128  # band width
CLAMP_MIN = 1e-12

_cache = {}


# ---------------------------------------------------------------- v5: band
def build_nc_band(n_xchunk=2, n_mask=4):
    nc = bacc.Bacc()
    xst = nc.declare_dram_parameter("xst", [D, NS], mybir.dt.bfloat16, isOutput=False)
    cb = nc.declare_dram_parameter("cb", [D, T * W], mybir.dt.bfloat16, isOutput=False)
    # small: cols [0:T] = rel, [T:T+8] = cnt (all values < 256, bf16-exact)
    small_in = nc.declare_dram_parameter(
        "small", [P, T + C // P], mybir.dt.bfloat16, isOutput=False
    )
    centers = nc.declare_dram_parameter(
        "centers", [C, D], mybir.dt.bfloat16, isOutput=False
    )
    out = nc.declare_dram_parameter("out", [1, 1], mybir.dt.float32, isOutput=True)

    c_t = centers.rearrange("(u p) d -> p u d", p=P)  # [128, 8, 128]
    UC = C // P  # 8

    with tile.TileContext(nc) as tc:
        with (
            tc.tile_pool(name="data", bufs=1) as data,
            tc.tile_pool(name="small", bufs=1) as small,
            tc.tile_pool(name="psum", bufs=4, space="PSUM") as psump,
            tc.tile_pool(name="psumr", bufs=1, space="PSUM") as psumr,
        ):
            x_sb = data.tile([P, NS], mybir.dt.bfloat16)
            cb_sb = data.tile([P, T, W], mybir.dt.bfloat16)
            oh_sb = data.tile([P, T, W], mybir.dt.bfloat16)
            mk_sb = data.tile([P, T, W], mybir.dt.bfloat16)
            c_sb = data.tile([P, UC, D], mybir.dt.bfloat16)
            cw_sb = data.tile([P, UC, D], mybir.dt.bfloat16)
            xsq_sb = data.tile([P, NS], mybir.dt.bfloat16)
            sm_sb = small.tile([P, T + UC], mybir.dt.bfloat16)
            csq_sb = small.tile([P, UC], mybir.dt.float32)
            iota_i = small.tile([P, W], mybir.dt.int32)
            iota_f = small.tile([P, W], mybir.dt.bfloat16)
            # accumulator columns: [0:n_xchunk] = x^2, [n_xchunk] = n*csq
            n_acc = n_xchunk + 1
            acc = small.tile([P, n_acc], mybir.dt.float32)
            tmp8 = small.tile([P, UC], mybir.dt.float32)
            ones = small.tile([P, 1], mybir.dt.float32)
            ones_bf = small.tile([P, 1], mybir.dt.bfloat16)
            fin = small.tile([1, n_acc + 1], mybir.dt.float32)
            res = small.tile([1, 1], mybir.dt.float32)

            nc.vector.memset(ones[:], 1.0)
            # the cross ones-vector carries the -2 weight of the cross term
            nc.vector.memset(ones_bf[:], -2.0)
            nc.gpsimd.iota(iota_i[:], pattern=[[1, W]], base=0, channel_multiplier=0)
            nc.vector.tensor_copy(out=iota_f[:], in_=iota_i[:])

            # DMA layout: sync ring carries x in 4 quarters; scalar ring
            # carries cb (2 halves) + small + centers. Finer chunks give
            # earlier completion sems for the dependent compute.
            cb_r = cb[:, :].rearrange("p (t w) -> p t w", w=W)
            h = T // 2
            q = NS // 4
            nc.scalar.dma_start(out=cb_sb[:, :h, :], in_=cb_r[:, :h, :])
            nc.scalar.dma_start(out=sm_sb[:], in_=small_in[:, :])
            for jq in range(4):
                nc.sync.dma_start(
                    out=x_sb[:, jq * q : (jq + 1) * q],
                    in_=xst[:, jq * q : (jq + 1) * q],
                )
            nc.scalar.dma_start(out=c_sb[:], in_=c_t[:, :, :])
            nc.scalar.dma_start(out=cb_sb[:, h:, :], in_=cb_r[:, h:, :])
            rel_sb = sm_sb[:, 0:T]
            cnt_sb = sm_sb[:, T : T + UC]
            # one-hot: oh[p, t, w] = (iota[w] == rel[p, t]); two chunks so
            # the first mask can start as soon as the first psum is ready
            hh = T // 2
            for c0, c1 in ((0, hh), (hh, T)):
                nc.vector.tensor_tensor(
                    out=oh_sb[:, c0:c1, :],
                    in0=iota_f[:, None, :].to_broadcast([P, c1 - c0, W]),
                    in1=rel_sb[:, c0:c1, None].to_broadcast([P, c1 - c0, W]),
                    op=mybir.AluOpType.is_equal,
                )
            # band matmuls: dot[s, w] = sum_d x[d, s] * cb[d, w]
            # one pool tile (= one PSUM bank) per mask group so PE writes of
            # group k+1 don't serialize against DVE reads of group k.
            # The masked chunks are partition-reduced by PE ones-matmuls
            # accumulating into a single [1, tpm*W] PSUM bank.
            tpm = T // n_mask
            psum_cr = psumr.tile([1, tpm * W], mybir.dt.float32, tag="cross")
            for k in range(n_mask):
                psum_k = psump.tile([P, tpm, W], mybir.dt.float32, tag="band")
                for i in range(tpm):
                    t = k * tpm + i
                    nc.tensor.matmul(
                        out=psum_k[:, i, :],
                        lhsT=x_sb[:, t * P : (t + 1) * P],
                        rhs=cb_sb[:, t, :],
                        start=True,
                        stop=True,
                    )
                ts = slice(k * tpm, (k + 1) * tpm)
                nc.vector.tensor_tensor(
                    out=mk_sb[:, ts, :],
                    in0=oh_sb[:, ts, :],
                    in1=psum_k[:, :, :],
                    op=mybir.AluOpType.mult,
                )
                nc.tensor.matmul(
                    out=psum_cr[:, :],
                    lhsT=ones_bf[:],
                    rhs=mk_sb[:, ts, :].rearrange("p t w -> p (t w)"),
                    start=(k == 0),
                    stop=(k == n_mask - 1),
                )
            # ||x||^2 chunks on ACT (bf16 in, fp32 accumulate); writes go to
            # a scratch tile so they don't serialize against the matmuls
            spx = NS // n_xchunk
            for j in range(n_xchunk):
                xs = slice(j * spx, (j + 1) * spx)
                nc.scalar.activation(
                    out=xsq_sb[:, xs],
                    in_=x_sb[:, xs],
                    func=mybir.ActivationFunctionType.Square,
                    accum_out=acc[:, j : j + 1],
                )
            # n_c * ||c_c||^2: gpsimd (idle) squares centers, DVE reduces
            nc.gpsimd.tensor_tensor(
                out=cw_sb[:], in0=c_sb[:], in1=c_sb[:], op=mybir.AluOpType.mult
            )
            nc.vector.reduce_sum(
                out=csq_sb[:], in_=cw_sb[:], axis=mybir.AxisListType.X
            )
            nc.vector.tensor_tensor(
                out=tmp8[:], in0=cnt_sb[:], in1=csq_sb[:], op=mybir.AluOpType.mult
            )
            nc.vector.reduce_sum(
                out=acc[:, n_xchunk : n_xchunk + 1],
                in_=tmp8[:],
                axis=mybir.AxisListType.X,
            )
            # final: partition-reduce acc via ones-matmul, then combine with
            # the cross sum: res = sum(acc cols) - 2 * sum(psum_cr)
            psum_fin = psumr.tile([1, n_acc], mybir.dt.float32, tag="fin")
            nc.tensor.matmul(
                out=psum_fin[:, :], lhsT=ones[:], rhs=acc[:], start=True, stop=True
            )
            nc.vector.reduce_sum(
                out=fin[:1, n_acc : n_acc + 1],
                in_=psum_cr[:1, :],
                axis=mybir.AxisListType.X,
            )
            nc.vector.tensor_copy(out=fin[:1, 0:n_acc], in_=psum_fin[:1, :])
            nc.vector.reduce_sum(
                out=res[:1, :1], in_=fin[:1, :], axis=mybir.AxisListType.X
            )
            nc.sync.dma_start(out=out[:, :], in_=res[:1, :1])
    nc.compile()
    return nc


def prep_band_core(x_shard, labels_shard, ct):
    """Host layout prep for one core. Returns in_map or None if a tile span
    exceeds the band width."""
    import ml_dtypes

    order = np.argsort(labels_shard, kind="stable")
    ls = labels_shard[order].astype(np.int64)
    bases = np.minimum(ls[::P][:T], C - W)  # [T]
    rel = ls.reshape(T, P).T - bases[None, :]  # [128, T]
    if rel.min() < 0 or rel.max() >= W:
        return None
    xs = x_shard[order]  # [NS, D]
    cb = np.concatenate([ct[:, b : b + W] for b in bases], axis=1)  # [D, T*W]
    cnt = np.bincount(labels_shard.astype(np.int64), minlength=C).astype(np.float32)
    small = np.concatenate(
        [rel.astype(np.float32), cnt.reshape(C // P, P).T], axis=1
    )
    return {
        "xst": np.ascontiguousarray(xs.T.astype(ml_dtypes.bfloat16)),
        "cb": np.ascontiguousarray(cb.astype(ml_dtypes.bfloat16)),
        "small": np.ascontiguousarray(small.astype(ml_dtypes.bfloat16)),
        "centers": None,  # filled by caller
    }


# ------------------------------------------------- v4: indirect-DMA gather
def build_nc_gather(n_chunk=4, n_xdma=4):
    nc = bacc.Bacc()
    x = nc.declare_dram_parameter("x", [NS, D], mybir.dt.float32, isOutput=False)
    centers = nc.declare_dram_parameter(
        "centers", [C, D], mybir.dt.float32, isOutput=False
    )
    labels = nc.declare_dram_parameter("labels", [P, T], mybir.dt.int32, isOutput=False)
    out = nc.declare_dram_parameter("out", [1, 1], mybir.dt.float32, isOutput=True)

    x_t = x.rearrange("(t p) d -> p t d", p=P)
    tpc = T // n_chunk

    with tile.TileContext(nc) as tc:
        with (
            tc.tile_pool(name="data", bufs=1) as data,
            tc.tile_pool(name="small", bufs=1) as small,
            tc.tile_pool(name="psum", bufs=1, space="PSUM") as psump,
        ):
            x_sb = data.tile([P, T, D], mybir.dt.float32)
            g_sb = data.tile([P, T, D], mybir.dt.float32)
            d_sb = data.tile([P, T, D], mybir.dt.float32)
            i_sb = small.tile([P, T], mybir.dt.int32)
            acc = small.tile([P, n_chunk], mybir.dt.float32)
            ones = small.tile([P, 1], mybir.dt.float32)

            nc.vector.memset(ones[:], 1.0)
            nc.sync.dma_start(out=i_sb[:], in_=labels[:, :])
            tpx = T // n_xdma
            for j in range(n_xdma):
                xs = slice(j * tpx, (j + 1) * tpx)
                nc.sync.dma_start(out=x_sb[:, xs, :], in_=x_t[:, xs, :])
            for t in range(T):
                nc.gpsimd.indirect_dma_start(
                    out=g_sb[:, t, :],
                    out_offset=None,
                    in_=centers[:],
                    in_offset=bass.IndirectOffsetOnAxis(ap=i_sb[:, t : t + 1], axis=0),
                )
            for k in range(n_chunk):
                ts = slice(k * tpc, (k + 1) * tpc)
                nc.vector.tensor_tensor(
                    out=d_sb[:, ts, :],
                    in0=x_sb[:, ts, :],
                    in1=g_sb[:, ts, :],
                    op=mybir.AluOpType.subtract,
                )
                nc.scalar.activation(
                    out=d_sb[:, ts, :],
                    in_=d_sb[:, ts, :],
                    func=mybir.ActivationFunctionType.Square,
                    accum_out=acc[:, k : k + 1],
                )
            psum = psump.tile([1, n_chunk], mybir.dt.float32)
            nc.tensor.matmul(
                out=psum[:, :], lhsT=ones[:], rhs=acc[:], start=True, stop=True
            )
            res = small.tile([1, 1], mybir.dt.float32)
            nc.vector.reduce_sum(
                out=res[:1, :1], in_=psum[:1, :], axis=mybir.AxisListType.X
            )
            nc.sync.dma_start(out=out[:, :], in_=res[:1, :1])
    nc.compile()
    return nc


# ----------------------------------------------------------------- driver
def make_in_maps(x, centers, labels):
    """Returns (in_maps, which) where which is 'band' or 'gather'."""
    x = np.ascontiguousarray(np.asarray(x, dtype=np.float32))
    centers = np.ascontiguousarray(np.asarray(centers, dtype=np.float32))
    labels = np.asarray(labels)
    ct = np.ascontiguousarray(centers.T)
    in_maps = []
    import ml_dtypes

    centers_bf = np.ascontiguousarray(centers.astype(ml_dtypes.bfloat16))
    for c in range(N_CORES):
        sl = slice(c * NS, (c + 1) * NS)
        m = prep_band_core(x[sl], labels[sl], ct)
        if m is None:
            break
        m["centers"] = centers_bf
        in_maps.append(m)
    else:
        return in_maps, "band"
    # fallback: indirect gather kernel
    in_maps = []
    for c in range(N_CORES):
        sl = slice(c * NS, (c + 1) * NS)
        in_maps.append(
            {
                "x": x[sl],
                "centers": centers,
                "labels": np.ascontiguousarray(
                    labels[sl].reshape(T, P).T.astype(np.int32)
                ),
            }
        )
    return in_maps, "gather"


def _get_nc(which):
    if which not in _cache:
        _cache[which] = (
            build_nc_band() if which == "band" else build_nc_gather()
        )
    return _cache[which]


def finalize(results):
    total = sum(float(results[c]["out"][0, 0]) for c in range(N_CORES))
    total += N * (C - 1) * CLAMP_MIN
    return np.float32(total / N)


def kernel(x, centers, labels):
    in_maps, which = make_in_maps(x, centers, labels)
    nc = _get_nc(which)
    res = run_bass_kernel_spmd(nc, in_maps, core_ids=list(range(N_CORES)))
    return finalize(res.results)
